# revision 18
# baseline (speedup 1.0000x reference)
"""Trainium2 Bass kernel for 3-layer ConvLSTM2D stack + BN/ReLU + Conv3D+sigmoid.

Model (per reference):
  for l in 0..2:  h = bn_relu(conv_lstm(h, k_l, rk_l, b_l), g_l, be_l)
  y = sigmoid(conv3d(h, w3) + b3)

Shapes: x [B=8, T=12, 64, 64, 1], F=64 filters per layer, 3x3 kernels, SAME.
Sharding: data-parallel over B across 8 NeuronCores (1 image/core).

v2 design:
- All matmuls bf16 (weights + moving data); gate math stays f32 in PSUM/SBUF.
- Inter-layer activations live in SBUF as 12 persistent frame tiles F_t
  [128, FSTRIDE] in "A-layout": upper 64:128 = frame, lower 0:64 = frame
  shifted +1 px (replica via SBUF-SBUF DMA).  3x3 conv = shifted-tap matmuls
  with 2-tap K-packing: 3 A-pairs (K=128) + 1 B-pair (K=128, from a B-layout
  tile: lower = frame, upper = frame shifted +66) + 1 A-single (K=64 upper).
  Recurrent path reads h A-tiles: 3 A-pairs + 3 A-singles (no B tile).
- Partition crossing for i*g -> c's partitions via SBUF-SBUF DMA (no PSUM
  identity matmul).
- conv3d is fused into layer 2's timestep loop (frame t-1 processed during
  timestep t, M=3 dt-outputs scattered into a [36, HW] bf16 ring; final
  bd-matmul + sigmoid pass).
- DMA spread: nc.sync = h-replica + ig-reloc + imt; nc.gpsimd (SWDGE) =
  B-tile builds + F-replica; nc.scalar = scatter/y/weights.
"""
import sys
import os

_REPO = '/opt/trn_rl_repo'
if _REPO not in sys.path:
    sys.path.insert(0, _REPO)

import numpy as np  # noqa: E402
import ml_dtypes  # noqa: E402

T, H, W, F = 12, 64, 64, 64
B = 8
PW = H + 2                 # padded width = 66
FRAME = PW * PW            # 4356
FSTRIDE = FRAME + 4        # frame stride per channel = 4360
NCHUNK = 8
CROWS = H // NCHUNK        # 8 rows per chunk
CHUNK = CROWS * W          # 512 pixels per chunk
INTER = PW + 1             # interior base offset = 67
XLEAD = PW + 1
XLEN = XLEAD + T * FRAME + 160

BF16 = ml_dtypes.bfloat16


def _off(t):
    return (t[0] - 1) * PW + (t[1] - 1)


# Tap groups.  A-pairs: (ta, ta+(0,1)); B-pair: ((0,2),(1,2)); singles on A.
PAIRS_A = [((0, 0), (0, 1)), ((1, 0), (1, 1)), ((2, 0), (2, 1))]
PAIR_B = ((0, 2), (1, 2))
SINGLES = [(0, 2), (1, 2), (2, 2)]   # rec/conv3d singles (A-tile upper)
SINGLE_INP = (2, 2)                  # inp single when B-pair used

# group entry: (kind, base); kind: 'A128' pair on A-tile, 'B128' pair on
# B-tile, 'A64' single on A-tile upper.
INP_GROUPS = ([('A128', INTER + _off(p[0])) for p in PAIRS_A] +
              [('B128', INTER + _off(PAIR_B[0]))] +
              [('A64', INTER + _off(SINGLE_INP))])
REC_GROUPS = INP_GROUPS  # rec also uses a B-tile of h (built per timestep)
C3_GROUPS = ([('A128', INTER + _off(p[0])) for p in PAIRS_A] +
             [('A64', INTER + _off(t)) for t in SINGLES])

N_WTI = 2 + 2 * 10       # layer0: 2;  layers 1-2: 10 each
N_WTR = 3 * 10           # 5 groups x 2 m per layer


def pack_weights(ks, rks):
    """Pack conv weights into bf16 [128,128] tiles.

    A-pair tile: rows 64:128 = k[ta], rows 0:64 = k[tb]  (upper=frame tap,
    lower=+1-shifted tap).  B-pair: rows 0:64 = k[ta], 64:128 = k[tb].
    Single: rows 64:128 = k[t].
    Returns (wt_inp, wt_rec) stacked arrays; tile order per layer matches
    INP_GROUPS/REC_GROUPS x m(0,1).
    """
    def sl(k, t, m):
        return k[t[0], t[1], :, m * 128:(m + 1) * 128]

    ti, tr = [], []
    for layer in range(3):
        k, rk = ks[layer], rks[layer]
        if layer == 0:
            k9 = k.reshape(9, 4 * F)
            for m in range(2):
                w = np.zeros((128, 128), np.float32)
                w[0:9, :] = k9[:, m * 128:(m + 1) * 128]
                ti.append(w)
        else:
            for i, (ta, tb) in enumerate(PAIRS_A):
                for m in range(2):
                    w = np.zeros((128, 128), np.float32)
                    w[64:128, :] = sl(k, ta, m)
                    w[0:64, :] = sl(k, tb, m)
                    ti.append(w)
            for m in range(2):
                w = np.zeros((128, 128), np.float32)
                w[0:64, :] = sl(k, PAIR_B[0], m)
                w[64:128, :] = sl(k, PAIR_B[1], m)
                ti.append(w)
            for m in range(2):
                w = np.zeros((128, 128), np.float32)
                w[64:128, :] = sl(k, SINGLE_INP, m)
                ti.append(w)
        for i, (ta, tb) in enumerate(PAIRS_A):
            for m in range(2):
                w = np.zeros((128, 128), np.float32)
                w[64:128, :] = sl(rk, ta, m)
                w[0:64, :] = sl(rk, tb, m)
                tr.append(w)
        for m in range(2):
            w = np.zeros((128, 128), np.float32)
            w[0:64, :] = sl(rk, PAIR_B[0], m)
            w[64:128, :] = sl(rk, PAIR_B[1], m)
            tr.append(w)
        for m in range(2):
            w = np.zeros((128, 128), np.float32)
            w[64:128, :] = sl(rk, SINGLE_INP, m)
            tr.append(w)
    return np.stack(ti).astype(BF16), np.stack(tr).astype(BF16)


def pack_w3(w3):
    """conv3d weights -> bf16 [6,128,4] tiles (cols 0:3 = dt index m)."""
    w3 = w3[:, :, :, :, 0]  # [3(dt), 3, 3, 64]
    tiles = np.zeros((6, 128, 4), np.float32)
    for i, (ta, tb) in enumerate(PAIRS_A):
        for m in range(3):
            tiles[i, 64:128, m] = w3[m, ta[0], ta[1], :]
            tiles[i, 0:64, m] = w3[m, tb[0], tb[1], :]
    for i, t in enumerate(SINGLES):
        for m in range(3):
            tiles[3 + i, 64:128, m] = w3[m, t[0], t[1], :]
    return tiles.astype(BF16)


def build_nc(TT=T):
    import concourse.bass as bass
    import concourse.mybir as mybir
    import concourse.tile as tile
    from concourse import bacc

    F32, BF = mybir.dt.float32, mybir.dt.bfloat16
    AF = mybir.ActivationFunctionType

    nc = bacc.Bacc("TRN2", target_bir_lowering=False, debug=False,
                   num_devices=8)

    d_x = nc.dram_tensor("x_im", [1, XLEN], BF, kind="ExternalInput")
    d_wti = nc.dram_tensor("wti", [N_WTI, 128, 128], BF, kind="ExternalInput")
    d_wtr = nc.dram_tensor("wtr", [N_WTR, 128, 128], BF, kind="ExternalInput")
    d_w3 = nc.dram_tensor("w3t", [6, 128, 4], BF, kind="ExternalInput")
    d_bd = nc.dram_tensor("bd", [3 * TT, TT], BF, kind="ExternalInput")
    d_b = nc.dram_tensor("b_all", [3, 256], F32, kind="ExternalInput")
    d_gb = nc.dram_tensor("gb_all", [3, 64], F32, kind="ExternalInput")
    d_be = nc.dram_tensor("be_all", [3, 64], F32, kind="ExternalInput")
    d_b3 = nc.dram_tensor("b3b", [TT, 1], F32, kind="ExternalInput")
    d_y = nc.dram_tensor("y", [TT, H * W], F32, kind="ExternalOutput")
    DBG = bool(os.environ.get("KDBG"))
    d_dbg = [nc.dram_tensor(f"dbg{l}", [TT, 64, FRAME], BF,
                            kind="ExternalOutput") if DBG else None
             for l in range(3)]

    def sub_ap(tile_obj, p0, np_, free_off, free_dims):
        base = tile_obj[:]
        ps = base.ap[0][0]
        return bass.AP(base.tensor, base.offset + p0 * ps + free_off,
                       [[ps, np_]] + [list(d) for d in free_dims])

    def conv_rhs(tile_obj, p0, kparts, base, chunk):
        return sub_ap(tile_obj, p0, kparts, base + chunk * CROWS * PW,
                      [[PW, CROWS], [1, W]])

    def interior_ap(tile_obj, chunk, p0=64):
        return sub_ap(tile_obj, p0, 64, INTER + chunk * CROWS * PW,
                      [[PW, CROWS], [1, W]])

    with tile.TileContext(nc) as tc:
        with tc.tile_pool(name="persist", bufs=1) as per, \
             tc.tile_pool(name="gates", bufs=2) as gates, \
             tc.tile_pool(name="ps", bufs=3, space="PSUM") as psp, \
             tc.tile_pool(name="pc3", bufs=2, space="PSUM") as pc3:

            # ---- biases / small constants ----
            b_if, b_go, gb_t, be_t = [], [], [], []
            for l in range(3):
                tt1 = per.tile([128, 1], F32, tag=f"bif{l}", name=f"bif{l}")
                nc.scalar.dma_start(
                    tt1[:], d_b[:][l, 0:128].rearrange("(a o) -> a o", o=1))
                b_if.append(tt1)
                tt2 = per.tile([128, 1], F32, tag=f"bgo{l}", name=f"bgo{l}")
                nc.scalar.dma_start(
                    tt2[:], d_b[:][l, 128:256].rearrange("(a o) -> a o", o=1))
                b_go.append(tt2)
                tt4 = per.tile([128, 1], F32, tag=f"gb{l}", name=f"gb{l}")
                nc.scalar.dma_start(
                    tt4[64:128, :],
                    d_gb[:][l, :].rearrange("(a o) -> a o", o=1))
                gb_t.append(tt4)
                tt5 = per.tile([128, 1], F32, tag=f"be{l}", name=f"be{l}")
                nc.scalar.dma_start(
                    tt5[64:128, :],
                    d_be[:][l, :].rearrange("(a o) -> a o", o=1))
                be_t.append(tt5)
            b3t = per.tile([TT, 1], F32, tag="b3", name="b3t")
            nc.scalar.dma_start(b3t[:], d_b3[:])
            bdt = per.tile([3 * TT, TT], BF, tag="bd", name="bdt")
            nc.scalar.dma_start(bdt[:], d_bd[:])

            # ---- weights ----
            wti_t, wtr_t = {}, {}
            for i in range(N_WTI):
                wt_ = per.tile([128, 128], BF, tag=f"wti{i}", name=f"wti{i}")
                nc.scalar.dma_start(wt_[:], d_wti[:][i, :, :])
                wti_t[i] = wt_
            for i in range(N_WTR):
                wt_ = per.tile([128, 128], BF, tag=f"wtr{i}", name=f"wtr{i}")
                nc.sync.dma_start(wt_[:], d_wtr[:][i, :, :])
                wtr_t[i] = wt_
            w3_t = []
            for i in range(6):
                w3i = per.tile([128, 4], BF, tag=f"w3_{i}", name=f"w3_{i}")
                nc.scalar.dma_start(w3i[:], d_w3[:][i, :, :])
                w3_t.append(w3i)

            # ---- persistent state ----
            Ft = [per.tile([128, FSTRIDE], BF, tag=f"F{t}", name=f"F{t}")
                  for t in range(TT)]
            hcur = [per.tile([128, FSTRIDE], BF, tag=f"h{i}", name=f"h{i}")
                    for i in range(2)]
            # B-tiles double as layer-0's 9-tap x tiles (partitions 0:9)
            Bt = [per.tile([128, FSTRIDE], BF, tag=f"B{i}", name=f"B{i}")
                  for i in range(2)]
            hB = [per.tile([128, FSTRIDE], BF, tag=f"hB{i}", name=f"hB{i}")
                  for i in range(2)]
            c_t = per.tile([128, H * W], F32, tag="c", name="c_t")
            ring = per.tile([3 * TT, H * W], BF, tag="ring", name="ring")

            # zero state once; frames/h only need padding zeroed
            def pad_memset(tile_obj, eng):
                eng.memset(tile_obj[:, 0:INTER], 0.0)
                eng.memset(sub_ap(tile_obj, 0, 128, PW + 65,
                                  [[PW, 63], [1, 2]]), 0.0)
                eng.memset(tile_obj[:, 64 * PW + 65:FSTRIDE], 0.0)

            for i, ftile in enumerate(Ft):
                pad_memset(ftile, nc.vector if i % 2 == 0 else nc.gpsimd)
            for hc in hcur:
                pad_memset(hc, nc.vector)
            for bt_ in Bt:
                nc.gpsimd.memset(bt_[:], 0.0)
            for hb_ in hB:
                nc.vector.memset(hb_[:], 0.0)
            nc.vector.memset(ring[:], 0.0)

            def load_imt(t):
                xap = d_x[:].rearrange("o n -> (o n)")
                src = bass.AP(xap.tensor, xap.offset + XLEAD + t * FRAME,
                              [[PW, 3], [1, 3], [1, FSTRIDE]])
                nc.sync.dma_start(sub_ap(Bt[t % 2], 0, 9, 0, [[1, FSTRIDE]]),
                                  src)

            def build_B(t):
                # B-layout of F[t]: lower = frame, upper = frame + PW
                dst = Bt[t % 2]
                src = Ft[t]
                nc.gpsimd.dma_start(
                    sub_ap(dst, 0, 64, 0, [[1, FRAME - PW]]),
                    sub_ap(src, 64, 64, 0, [[1, FRAME - PW]]))
                nc.gpsimd.dma_start(
                    sub_ap(dst, 64, 64, 0, [[1, FRAME - PW]]),
                    sub_ap(src, 64, 64, PW, [[1, FRAME - PW]]))

            def replica(tile_obj, eng, npiece=4):
                # lower 0:64 <- upper 64:128 shifted +1
                piece = FRAME // npiece
                for p in range(npiece):
                    eng.dma_start(
                        sub_ap(tile_obj, 0, 64, p * piece, [[1, piece]]),
                        sub_ap(tile_obj, 64, 64, p * piece + 1, [[1, piece]]))

            def build_hB(src_h, dstB, npiece=4):
                # B-layout of h: lower = h, upper = h + PW (reads bounded
                # by 4224 so 4*1072 coverage suffices)
                piece = 1072
                for p in range(npiece):
                    nc.sync.dma_start(
                        sub_ap(dstB, 0, 64, p * piece, [[1, piece]]),
                        sub_ap(src_h, 64, 64, p * piece, [[1, piece]]))
                    nc.sync.dma_start(
                        sub_ap(dstB, 64, 64, p * piece, [[1, piece]]),
                        sub_ap(src_h, 64, 64, p * piece + PW, [[1, piece]]))

            def conv3d_frame_chunk(tp, chunk):
                pP = pc3.tile([3, CHUNK], F32, tag="pP", name="pP")
                n3 = len(C3_GROUPS)
                # all matmuls K=128 full row-group (zero-padded weights);
                # quarter/half row-group configs stall the PE pipe
                for j, (kind, base) in enumerate(C3_GROUPS):
                    lhs = w3_t[j][0:128, 0:3]
                    rhs = conv_rhs(Ft[tp], 0, 128, base, chunk)
                    nc.tensor.matmul(pP[:], lhs, rhs, start=(j == 0),
                                     stop=(j == n3 - 1))
                pstf = gates.tile([3, CHUNK], BF, tag="pstf", name="pstf")
                nc.scalar.activation(pstf[:], pP[:], AF.Copy)
                # scatter rows m -> ring partition m*TT + (tp+1-m)
                ms = [m for m in range(3) if 0 <= tp + 1 - m < TT]
                m0, mn = ms[0], len(ms)
                rb = ring[:]
                dst = bass.AP(rb.tensor,
                              rb.offset + (m0 * TT + tp + 1 - m0) *
                              rb.ap[0][0] + chunk * CHUNK,
                              [[(TT - 1) * rb.ap[0][0], mn], [1, CHUNK]])
                psrc = pstf[:]
                srcp = bass.AP(psrc.tensor, psrc.offset + m0 * psrc.ap[0][0],
                               [[psrc.ap[0][0], mn], [1, CHUNK]])
                nc.scalar.dma_start(dst, srcp)

            # ================= ConvLSTM layers =================
            for layer in range(3):
                wt_base = 0 if layer == 0 else 2 + (layer - 1) * 10
                rec_base = layer * 10
                LAG = 2
                if layer == 0:
                    load_imt(0)
                else:
                    build_B(0)
                for t in range(TT):
                    hprev = hcur[(t + 1) % 2]
                    hnew = hcur[t % 2]
                    # prefetch next timestep's input tile
                    if t + 1 < TT:
                        if layer == 0:
                            load_imt(t + 1)
                        else:
                            build_B(t + 1)

                    pend = {}

                    def gate_front(chunk, psA, psB):
                        nc.scalar.activation(psA[:], psA[:], AF.Sigmoid,
                                             bias=b_if[layer][:])
                        g_t = gates.tile([64, CHUNK], BF, tag="g_t",
                                         name="g_t")
                        nc.scalar.activation(g_t[:], psB[0:64, :], AF.Tanh,
                                             bias=b_go[layer][0:64, :])
                        nc.scalar.activation(psB[64:128, :], psB[64:128, :],
                                             AF.Sigmoid,
                                             bias=b_go[layer][64:128, :])
                        ig = gates.tile([128, CHUNK], BF, tag="ig", name="ig",
                                        bufs=3)
                        nc.vector.tensor_mul(ig[0:64, :], psA[0:64, :],
                                             g_t[:])
                        nc.sync.dma_start(ig[64:128, :], ig[0:64, :])
                        return ig

                    def gate_back(chunk, psA, psB, ig):
                        csl = c_t[64:128, chunk * CHUNK:(chunk + 1) * CHUNK]
                        if t == 0:
                            nc.vector.tensor_copy(csl, ig[64:128, :])
                        else:
                            nc.vector.tensor_mul(csl, csl, psA[64:128, :])
                            nc.vector.tensor_add(csl, csl, ig[64:128, :])
                        tc128 = gates.tile([128, CHUNK], F32, tag="tc",
                                           name="tc", bufs=3)
                        nc.scalar.activation(tc128[64:128, :], csl, AF.Tanh)
                        nc.vector.tensor_mul(interior_ap(hnew, chunk),
                                             psB[64:128, :],
                                             tc128[64:128, :])
                        # BN + ReLU on GpSimd (Act engine is near-critical)
                        bnt = gates.tile([128, CHUNK], F32, tag="tc",
                                         name="bnt", bufs=3)
                        nc.gpsimd.tensor_scalar(
                            bnt[64:128, :], interior_ap(hnew, chunk),
                            gb_t[layer][64:128, :], be_t[layer][64:128, :],
                            mybir.AluOpType.mult, mybir.AluOpType.add)
                        nc.gpsimd.tensor_scalar_max(
                            interior_ap(Ft[t], chunk), bnt[64:128, :], 0.0)

                    for chunk in range(NCHUNK):
                        psA = psp.tile([128, CHUNK], F32, tag="psA",
                                       name="psA")
                        psB = psp.tile([128, CHUNK], F32, tag="psB",
                                       name="psB")
                        for m, pst in ((0, psA), (1, psB)):
                            mms = []
                            if layer == 0:
                                rhs = conv_rhs(Bt[t % 2], 0, 128, 0, chunk)
                                mms.append((wti_t[wt_base + m][0:128, :],
                                            rhs))
                            else:
                                for gi, (kind, base) in enumerate(INP_GROUPS):
                                    wi = wti_t[wt_base + 2 * gi + m]
                                    src = (Bt[t % 2] if kind == 'B128'
                                           else Ft[t])
                                    rhs = conv_rhs(src, 0, 128, base, chunk)
                                    mms.append((wi[0:128, :], rhs))
                            if t > 0:
                                mm_last = None
                                for gi, (kind, base) in enumerate(REC_GROUPS):
                                    wi = wtr_t[rec_base + 2 * gi + m]
                                    if kind == 'B128':
                                        # hB built late in t-1: order last
                                        rhs = conv_rhs(hB[(t + 1) % 2], 0,
                                                       128, base, chunk)
                                        mm_last = (wi[0:128, :], rhs)
                                    else:
                                        rhs = conv_rhs(hprev, 0, 128, base,
                                                       chunk)
                                        mms.append((wi[0:128, :], rhs))
                                mms.append(mm_last)
                            nmm = len(mms)
                            for j, (lw, rhs) in enumerate(mms):
                                nc.tensor.matmul(pst[:], lw, rhs,
                                                 start=(j == 0),
                                                 stop=(j == nmm - 1))
                        # interleave conv3d work for frame t-1 (layer 2)
                        if layer == 2 and t > 0:
                            conv3d_frame_chunk(t - 1, chunk)

                        ig = gate_front(chunk, psA, psB)
                        pend[chunk] = (psA, psB, ig)
                        if chunk - LAG in pend:
                            pA, pB, pig = pend.pop(chunk - LAG)
                            gate_back(chunk - LAG, pA, pB, pig)

                    for cc in sorted(pend):
                        pA, pB, pig = pend[cc]
                        gate_back(cc, pA, pB, pig)
                    pend.clear()

                    replica(hnew, nc.sync)
                    if t + 1 < TT:
                        build_hB(hnew, hB[t % 2])
                    replica(Ft[t], nc.gpsimd)
                    if DBG:
                        nc.scalar.dma_start(d_dbg[layer][:][t, :, :],
                                            Ft[t][64:128, 0:FRAME])

            # trailing conv3d frame (T-1)
            for chunk in range(NCHUNK):
                conv3d_frame_chunk(TT - 1, chunk)

            # ================= final: bd matmul + sigmoid =================
            for chunk in range(NCHUNK):
                pY = pc3.tile([TT, CHUNK], F32, tag="pP", name="pY")
                nc.tensor.matmul(
                    pY[:], bdt[:],
                    ring[:, chunk * CHUNK:(chunk + 1) * CHUNK],
                    start=True, stop=True)
                ystg = gates.tile([TT, CHUNK], F32, tag="ystg", name="ystg",
                                  bufs=1)
                nc.scalar.activation(ystg[:], pY[:], AF.Sigmoid, bias=b3t[:])
                nc.sync.dma_start(
                    d_y[:][:, chunk * CHUNK:(chunk + 1) * CHUNK], ystg[:])

    nc.compile()
    return nc


def prep_inputs(x, k0, rk0, b0, g0, be0, k1, rk1, b1, g1, be1,
                k2, rk2, b2, g2, be2, w3, b3, TT=T):
    x = np.asarray(x, np.float32)
    wti, wtr = pack_weights(
        [np.asarray(k0, np.float32), np.asarray(k1, np.float32),
         np.asarray(k2, np.float32)],
        [np.asarray(rk0, np.float32), np.asarray(rk1, np.float32),
         np.asarray(rk2, np.float32)])
    w3t = pack_w3(np.asarray(w3, np.float32))
    b_all = np.stack([np.asarray(b0, np.float32),
                      np.asarray(b1, np.float32),
                      np.asarray(b2, np.float32)])
    scale = np.float32(1.0 / np.sqrt(1.0 + 1e-3))
    gb_all = np.stack([np.asarray(g0, np.float32) * scale,
                       np.asarray(g1, np.float32) * scale,
                       np.asarray(g2, np.float32) * scale])
    be_all = np.stack([np.asarray(be0, np.float32),
                       np.asarray(be1, np.float32),
                       np.asarray(be2, np.float32)])
    bd = np.zeros((3 * TT, TT), np.float32)
    for m in range(3):
        for t in range(TT):
            bd[m * TT + t, t] = 1.0
    b3b = np.full((TT, 1), np.asarray(b3, np.float32).ravel()[0], np.float32)

    shared = dict(wti=wti, wtr=wtr, w3t=w3t, bd=bd.astype(BF16),
                  b_all=b_all, gb_all=gb_all, be_all=be_all, b3b=b3b)
    in_maps = []
    for bb in range(B):
        xi = np.zeros((1, XLEN), BF16)
        fr = np.zeros((TT, PW, PW), np.float32)
        fr[:, 1:H + 1, 1:W + 1] = x[bb, :TT, :, :, 0]
        xi[0, XLEAD:XLEAD + TT * FRAME] = fr.reshape(-1).astype(BF16)
        m = dict(shared)
        m["x_im"] = xi
        in_maps.append(m)
    return in_maps


_CACHED = {}


def kernel(**inputs):
    from concourse.bass_utils import run_bass_kernel_spmd
    if 'nc' not in _CACHED:
        _CACHED['nc'] = build_nc(T)
    nc = _CACHED['nc']
    in_maps = prep_inputs(**inputs)
    res = run_bass_kernel_spmd(nc, in_maps, core_ids=list(range(B)),
                               trace=bool(os.environ.get('KTRACE')))
    _CACHED['last_res'] = res
    y = np.stack([r["y"].reshape(T, H, W, 1) for r in res.results])
    return y


# revision 19
# speedup vs baseline: 1.9137x; 1.9137x over previous
"""Trainium2 Bass kernel for 3-layer ConvLSTM2D stack + BN/ReLU + Conv3D+sigmoid.

Model (per reference):
  for l in 0..2:  h = bn_relu(conv_lstm(h, k_l, rk_l, b_l), g_l, be_l)
  y = sigmoid(conv3d(h, w3) + b3)

Shapes: x [B=8, T=12, 64, 64, 1], F=64 filters per layer, 3x3 kernels, SAME.
Sharding: data-parallel over B across 8 NeuronCores (1 image/core).

v2 design:
- All matmuls bf16 (weights + moving data); gate math stays f32 in PSUM/SBUF.
- Inter-layer activations live in SBUF as 12 persistent frame tiles F_t
  [128, FSTRIDE] in "A-layout": upper 64:128 = frame, lower 0:64 = frame
  shifted +1 px (replica via SBUF-SBUF DMA).  3x3 conv = shifted-tap matmuls
  with 2-tap K-packing: 3 A-pairs (K=128) + 1 B-pair (K=128, from a B-layout
  tile: lower = frame, upper = frame shifted +66) + 1 A-single (K=64 upper).
  Recurrent path reads h A-tiles: 3 A-pairs + 3 A-singles (no B tile).
- Partition crossing for i*g -> c's partitions via SBUF-SBUF DMA (no PSUM
  identity matmul).
- conv3d is fused into layer 2's timestep loop (frame t-1 processed during
  timestep t, M=3 dt-outputs scattered into a [36, HW] bf16 ring; final
  bd-matmul + sigmoid pass).
- DMA spread: nc.sync = h-replica + ig-reloc + imt; nc.gpsimd (SWDGE) =
  B-tile builds + F-replica; nc.scalar = scatter/y/weights.
"""
import sys
import os

_REPO = '/opt/trn_rl_repo'
if _REPO not in sys.path:
    sys.path.insert(0, _REPO)

import numpy as np  # noqa: E402
import ml_dtypes  # noqa: E402

T, H, W, F = 12, 64, 64, 64
B = 8
PW = H + 2                 # padded width = 66
FRAME = PW * PW            # 4356
FSTRIDE = FRAME + 4        # frame stride per channel = 4360
NCHUNK = 8
CROWS = H // NCHUNK        # 8 rows per chunk
CHUNK = CROWS * W          # 512 pixels per chunk
INTER = PW + 1             # interior base offset = 67
XLEAD = PW + 1
XLEN = XLEAD + T * FRAME + 160

BF16 = ml_dtypes.bfloat16


def _off(t):
    return (t[0] - 1) * PW + (t[1] - 1)


# Tap groups.  A-pairs: (ta, ta+(0,1)); B-pair: ((0,2),(1,2)); singles on A.
PAIRS_A = [((0, 0), (0, 1)), ((1, 0), (1, 1)), ((2, 0), (2, 1))]
PAIR_B = ((0, 2), (1, 2))
SINGLES = [(0, 2), (1, 2), (2, 2)]   # rec/conv3d singles (A-tile upper)
SINGLE_INP = (2, 2)                  # inp single when B-pair used

# group entry: (kind, base); kind: 'A128' pair on A-tile, 'B128' pair on
# B-tile, 'A64' single on A-tile upper.
INP_GROUPS = ([('A128', INTER + _off(p[0])) for p in PAIRS_A] +
              [('B128', INTER + _off(PAIR_B[0]))] +
              [('A64', INTER + _off(SINGLE_INP))])
REC_GROUPS = INP_GROUPS  # rec also uses a B-tile of h (built per timestep)
C3_GROUPS = ([('A128', INTER + _off(p[0])) for p in PAIRS_A] +
             [('A64', INTER + _off(t)) for t in SINGLES])

N_WTI = 2 + 2 * 10       # layer0: 2;  layers 1-2: 10 each
N_WTR = 3 * 10           # 5 groups x 2 m per layer


def pack_weights(ks, rks):
    """Pack conv weights into bf16 [128,128] tiles.

    A-pair tile: rows 64:128 = k[ta], rows 0:64 = k[tb]  (upper=frame tap,
    lower=+1-shifted tap).  B-pair: rows 0:64 = k[ta], 64:128 = k[tb].
    Single: rows 64:128 = k[t].
    Returns (wt_inp, wt_rec) stacked arrays; tile order per layer matches
    INP_GROUPS/REC_GROUPS x m(0,1).
    """
    def sl(k, t, m):
        return k[t[0], t[1], :, m * 128:(m + 1) * 128]

    ti, tr = [], []
    for layer in range(3):
        k, rk = ks[layer], rks[layer]
        if layer == 0:
            k9 = k.reshape(9, 4 * F)
            for m in range(2):
                w = np.zeros((128, 128), np.float32)
                w[0:9, :] = k9[:, m * 128:(m + 1) * 128]
                ti.append(w)
        else:
            for i, (ta, tb) in enumerate(PAIRS_A):
                for m in range(2):
                    w = np.zeros((128, 128), np.float32)
                    w[64:128, :] = sl(k, ta, m)
                    w[0:64, :] = sl(k, tb, m)
                    ti.append(w)
            for m in range(2):
                w = np.zeros((128, 128), np.float32)
                w[0:64, :] = sl(k, PAIR_B[0], m)
                w[64:128, :] = sl(k, PAIR_B[1], m)
                ti.append(w)
            for m in range(2):
                w = np.zeros((128, 128), np.float32)
                w[64:128, :] = sl(k, SINGLE_INP, m)
                ti.append(w)
        for i, (ta, tb) in enumerate(PAIRS_A):
            for m in range(2):
                w = np.zeros((128, 128), np.float32)
                w[64:128, :] = sl(rk, ta, m)
                w[0:64, :] = sl(rk, tb, m)
                tr.append(w)
        for m in range(2):
            w = np.zeros((128, 128), np.float32)
            w[0:64, :] = sl(rk, PAIR_B[0], m)
            w[64:128, :] = sl(rk, PAIR_B[1], m)
            tr.append(w)
        for m in range(2):
            w = np.zeros((128, 128), np.float32)
            w[64:128, :] = sl(rk, SINGLE_INP, m)
            tr.append(w)
    return np.stack(ti).astype(BF16), np.stack(tr).astype(BF16)


def pack_w3(w3):
    """conv3d weights -> bf16 [6,128,4] tiles (cols 0:3 = dt index m)."""
    w3 = w3[:, :, :, :, 0]  # [3(dt), 3, 3, 64]
    tiles = np.zeros((6, 128, 4), np.float32)
    for i, (ta, tb) in enumerate(PAIRS_A):
        for m in range(3):
            tiles[i, 64:128, m] = w3[m, ta[0], ta[1], :]
            tiles[i, 0:64, m] = w3[m, tb[0], tb[1], :]
    for i, t in enumerate(SINGLES):
        for m in range(3):
            tiles[3 + i, 64:128, m] = w3[m, t[0], t[1], :]
    return tiles.astype(BF16)


def build_nc(TT=T):
    import concourse.bass as bass
    import concourse.mybir as mybir
    import concourse.tile as tile
    from concourse import bacc

    F32, BF = mybir.dt.float32, mybir.dt.bfloat16
    AF = mybir.ActivationFunctionType

    nc = bacc.Bacc("TRN2", target_bir_lowering=False, debug=False,
                   num_devices=8)

    d_x = nc.dram_tensor("x_im", [1, XLEN], BF, kind="ExternalInput")
    d_wti = nc.dram_tensor("wti", [N_WTI, 128, 128], BF, kind="ExternalInput")
    d_wtr = nc.dram_tensor("wtr", [N_WTR, 128, 128], BF, kind="ExternalInput")
    d_w3 = nc.dram_tensor("w3t", [6, 128, 4], BF, kind="ExternalInput")
    d_bd = nc.dram_tensor("bd", [3 * TT, TT], BF, kind="ExternalInput")
    d_b = nc.dram_tensor("b_all", [3, 256], F32, kind="ExternalInput")
    d_gb = nc.dram_tensor("gb_all", [3, 64], F32, kind="ExternalInput")
    d_be = nc.dram_tensor("be_all", [3, 64], F32, kind="ExternalInput")
    d_b3 = nc.dram_tensor("b3b", [TT, 1], F32, kind="ExternalInput")
    d_y = nc.dram_tensor("y", [TT, H * W], F32, kind="ExternalOutput")
    DBG = bool(os.environ.get("KDBG"))
    d_dbg = [nc.dram_tensor(f"dbg{l}", [TT, 64, FRAME], BF,
                            kind="ExternalOutput") if DBG else None
             for l in range(3)]

    def sub_ap(tile_obj, p0, np_, free_off, free_dims):
        base = tile_obj[:]
        ps = base.ap[0][0]
        return bass.AP(base.tensor, base.offset + p0 * ps + free_off,
                       [[ps, np_]] + [list(d) for d in free_dims])

    def conv_rhs(tile_obj, p0, kparts, base, chunk):
        return sub_ap(tile_obj, p0, kparts, base + chunk * CROWS * PW,
                      [[PW, CROWS], [1, W]])

    def interior_ap(tile_obj, chunk, p0=64):
        return sub_ap(tile_obj, p0, 64, INTER + chunk * CROWS * PW,
                      [[PW, CROWS], [1, W]])

    with tile.TileContext(nc) as tc:
        with tc.tile_pool(name="persist", bufs=1) as per, \
             tc.tile_pool(name="gates", bufs=2) as gates, \
             tc.tile_pool(name="ps", bufs=3, space="PSUM") as psp, \
             tc.tile_pool(name="pc3", bufs=2, space="PSUM") as pc3:

            # ---- biases / small constants ----
            b_if, b_go, gb_t, be_t = [], [], [], []
            for l in range(3):
                tt1 = per.tile([128, 1], F32, tag=f"bif{l}", name=f"bif{l}")
                nc.scalar.dma_start(
                    tt1[:], d_b[:][l, 0:128].rearrange("(a o) -> a o", o=1))
                b_if.append(tt1)
                tt2 = per.tile([128, 1], F32, tag=f"bgo{l}", name=f"bgo{l}")
                nc.scalar.dma_start(
                    tt2[:], d_b[:][l, 128:256].rearrange("(a o) -> a o", o=1))
                b_go.append(tt2)
                tt4 = per.tile([128, 1], F32, tag=f"gb{l}", name=f"gb{l}")
                nc.scalar.dma_start(
                    tt4[64:128, :],
                    d_gb[:][l, :].rearrange("(a o) -> a o", o=1))
                gb_t.append(tt4)
                tt5 = per.tile([128, 1], F32, tag=f"be{l}", name=f"be{l}")
                nc.scalar.dma_start(
                    tt5[64:128, :],
                    d_be[:][l, :].rearrange("(a o) -> a o", o=1))
                be_t.append(tt5)
            b3t = per.tile([TT, 1], F32, tag="b3", name="b3t")
            nc.scalar.dma_start(b3t[:], d_b3[:])
            bdt = per.tile([3 * TT, TT], BF, tag="bd", name="bdt")
            nc.scalar.dma_start(bdt[:], d_bd[:])

            # ---- weights ----
            wti_t, wtr_t = {}, {}
            for i in range(N_WTI):
                wt_ = per.tile([128, 128], BF, tag=f"wti{i}", name=f"wti{i}")
                nc.scalar.dma_start(wt_[:], d_wti[:][i, :, :])
                wti_t[i] = wt_
            for i in range(N_WTR):
                wt_ = per.tile([128, 128], BF, tag=f"wtr{i}", name=f"wtr{i}")
                nc.sync.dma_start(wt_[:], d_wtr[:][i, :, :])
                wtr_t[i] = wt_
            w3_t = []
            for i in range(6):
                w3i = per.tile([128, 4], BF, tag=f"w3_{i}", name=f"w3_{i}")
                nc.scalar.dma_start(w3i[:], d_w3[:][i, :, :])
                w3_t.append(w3i)

            # ---- persistent state ----
            Ft = [per.tile([128, FSTRIDE], BF, tag=f"F{t}", name=f"F{t}")
                  for t in range(TT)]
            hcur = [per.tile([128, FSTRIDE], BF, tag=f"h{i}", name=f"h{i}")
                    for i in range(2)]
            # B-tiles double as layer-0's 9-tap x tiles (partitions 0:9)
            Bt = [per.tile([128, FSTRIDE], BF, tag=f"B{i}", name=f"B{i}")
                  for i in range(2)]
            hB = [per.tile([128, FSTRIDE], BF, tag=f"hB{i}", name=f"hB{i}")
                  for i in range(2)]
            c_t = per.tile([128, H * W], F32, tag="c", name="c_t")
            ring = per.tile([3 * TT, H * W], BF, tag="ring", name="ring")

            # zero state once; frames/h only need padding zeroed
            def pad_memset(tile_obj, eng):
                eng.memset(tile_obj[:, 0:INTER], 0.0)
                eng.memset(sub_ap(tile_obj, 0, 128, PW + 65,
                                  [[PW, 63], [1, 2]]), 0.0)
                eng.memset(tile_obj[:, 64 * PW + 65:FSTRIDE], 0.0)

            for i, ftile in enumerate(Ft):
                pad_memset(ftile, nc.vector if i % 2 == 0 else nc.gpsimd)
            for hc in hcur:
                pad_memset(hc, nc.vector)
            for bt_ in Bt:
                nc.gpsimd.memset(bt_[:], 0.0)
            for hb_ in hB:
                nc.vector.memset(hb_[:], 0.0)
            nc.vector.memset(ring[:], 0.0)

            def load_imt(t):
                xap = d_x[:].rearrange("o n -> (o n)")
                src = bass.AP(xap.tensor, xap.offset + XLEAD + t * FRAME,
                              [[PW, 3], [1, 3], [1, FSTRIDE]])
                nc.sync.dma_start(sub_ap(Bt[t % 2], 0, 9, 0, [[1, FSTRIDE]]),
                                  src)

            def build_B(t):
                # B-layout of F[t]: lower = frame, upper = frame + PW
                dst = Bt[t % 2]
                src = Ft[t]
                nc.gpsimd.dma_start(
                    sub_ap(dst, 0, 64, 0, [[1, FRAME - PW]]),
                    sub_ap(src, 64, 64, 0, [[1, FRAME - PW]]))
                nc.gpsimd.dma_start(
                    sub_ap(dst, 64, 64, 0, [[1, FRAME - PW]]),
                    sub_ap(src, 64, 64, PW, [[1, FRAME - PW]]))

            def replica(tile_obj, eng, npiece=4):
                # lower 0:64 <- upper 64:128 shifted +1
                piece = FRAME // npiece
                for p in range(npiece):
                    eng.dma_start(
                        sub_ap(tile_obj, 0, 64, p * piece, [[1, piece]]),
                        sub_ap(tile_obj, 64, 64, p * piece + 1, [[1, piece]]))

            def build_hB(src_h, dstB, npiece=4):
                # B-layout of h: lower = h, upper = h + PW (reads bounded
                # by 4224 so 4*1072 coverage suffices)
                piece = 1072
                for p in range(npiece):
                    nc.sync.dma_start(
                        sub_ap(dstB, 0, 64, p * piece, [[1, piece]]),
                        sub_ap(src_h, 64, 64, p * piece, [[1, piece]]))
                    nc.sync.dma_start(
                        sub_ap(dstB, 64, 64, p * piece, [[1, piece]]),
                        sub_ap(src_h, 64, 64, p * piece + PW, [[1, piece]]))

            def conv3d_frame_chunk(tp, chunk):
                pP = pc3.tile([3, CHUNK], F32, tag="pP", name="pP")
                n3 = len(C3_GROUPS)
                # all matmuls K=128 full row-group (zero-padded weights);
                # quarter/half row-group configs stall the PE pipe
                for j, (kind, base) in enumerate(C3_GROUPS):
                    lhs = w3_t[j][0:128, 0:3]
                    rhs = conv_rhs(Ft[tp], 0, 128, base, chunk)
                    nc.tensor.matmul(pP[:], lhs, rhs, start=(j == 0),
                                     stop=(j == n3 - 1))
                pstf = gates.tile([3, CHUNK], BF, tag="pstf", name="pstf")
                nc.scalar.activation(pstf[:], pP[:], AF.Copy)
                # scatter rows m -> ring partition m*TT + (tp+1-m)
                ms = [m for m in range(3) if 0 <= tp + 1 - m < TT]
                m0, mn = ms[0], len(ms)
                rb = ring[:]
                dst = bass.AP(rb.tensor,
                              rb.offset + (m0 * TT + tp + 1 - m0) *
                              rb.ap[0][0] + chunk * CHUNK,
                              [[(TT - 1) * rb.ap[0][0], mn], [1, CHUNK]])
                psrc = pstf[:]
                srcp = bass.AP(psrc.tensor, psrc.offset + m0 * psrc.ap[0][0],
                               [[psrc.ap[0][0], mn], [1, CHUNK]])
                nc.scalar.dma_start(dst, srcp)

            # ================= ConvLSTM layers =================
            for layer in range(3):
                wt_base = 0 if layer == 0 else 2 + (layer - 1) * 10
                rec_base = layer * 10
                LAG = 2
                if layer == 0:
                    load_imt(0)
                else:
                    build_B(0)
                for t in range(TT):
                    hprev = hcur[(t + 1) % 2]
                    hnew = hcur[t % 2]
                    # prefetch next timestep's input tile
                    if t + 1 < TT:
                        if layer == 0:
                            load_imt(t + 1)
                        else:
                            build_B(t + 1)

                    pend = {}

                    def gate_front(chunk, psA, psB):
                        nc.scalar.activation(psA[:], psA[:], AF.Sigmoid,
                                             bias=b_if[layer][:])
                        g_t = gates.tile([64, CHUNK], BF, tag="g_t",
                                         name="g_t")
                        nc.scalar.activation(g_t[:], psB[0:64, :], AF.Tanh,
                                             bias=b_go[layer][0:64, :])
                        nc.scalar.activation(psB[64:128, :], psB[64:128, :],
                                             AF.Sigmoid,
                                             bias=b_go[layer][64:128, :])
                        ig = gates.tile([128, CHUNK], BF, tag="ig", name="ig",
                                        bufs=3)
                        nc.vector.tensor_mul(ig[0:64, :], psA[0:64, :],
                                             g_t[:])
                        nc.sync.dma_start(ig[64:128, :], ig[0:64, :])
                        return ig

                    def gate_back(chunk, psA, psB, ig):
                        csl = c_t[64:128, chunk * CHUNK:(chunk + 1) * CHUNK]
                        if t == 0:
                            nc.vector.tensor_copy(csl, ig[64:128, :])
                        else:
                            nc.vector.tensor_mul(csl, csl, psA[64:128, :])
                            nc.vector.tensor_add(csl, csl, ig[64:128, :])
                        tc128 = gates.tile([128, CHUNK], F32, tag="tc",
                                           name="tc", bufs=3)
                        nc.scalar.activation(tc128[64:128, :], csl, AF.Tanh)
                        nc.vector.tensor_mul(interior_ap(hnew, chunk),
                                             psB[64:128, :],
                                             tc128[64:128, :])
                        nc.scalar.activation(
                            interior_ap(Ft[t], chunk),
                            interior_ap(hnew, chunk), AF.Relu,
                            bias=be_t[layer][64:128, :],
                            scale=gb_t[layer][64:128, :])

                    for chunk in range(NCHUNK):
                        psA = psp.tile([128, CHUNK], F32, tag="psA",
                                       name="psA")
                        psB = psp.tile([128, CHUNK], F32, tag="psB",
                                       name="psB")
                        for m, pst in ((0, psA), (1, psB)):
                            mms = []
                            if layer == 0:
                                rhs = conv_rhs(Bt[t % 2], 0, 128, 0, chunk)
                                mms.append((wti_t[wt_base + m][0:128, :],
                                            rhs))
                            else:
                                for gi, (kind, base) in enumerate(INP_GROUPS):
                                    wi = wti_t[wt_base + 2 * gi + m]
                                    src = (Bt[t % 2] if kind == 'B128'
                                           else Ft[t])
                                    rhs = conv_rhs(src, 0, 128, base, chunk)
                                    mms.append((wi[0:128, :], rhs))
                            if t > 0:
                                mm_last = None
                                for gi, (kind, base) in enumerate(REC_GROUPS):
                                    wi = wtr_t[rec_base + 2 * gi + m]
                                    if kind == 'B128':
                                        # hB built late in t-1: order last
                                        rhs = conv_rhs(hB[(t + 1) % 2], 0,
                                                       128, base, chunk)
                                        mm_last = (wi[0:128, :], rhs)
                                    else:
                                        rhs = conv_rhs(hprev, 0, 128, base,
                                                       chunk)
                                        mms.append((wi[0:128, :], rhs))
                                mms.append(mm_last)
                            nmm = len(mms)
                            for j, (lw, rhs) in enumerate(mms):
                                nc.tensor.matmul(pst[:], lw, rhs,
                                                 start=(j == 0),
                                                 stop=(j == nmm - 1))
                        # interleave conv3d work for frame t-1 (layer 2)
                        if layer == 2 and t > 0:
                            conv3d_frame_chunk(t - 1, chunk)

                        ig = gate_front(chunk, psA, psB)
                        pend[chunk] = (psA, psB, ig)
                        if chunk - LAG in pend:
                            pA, pB, pig = pend.pop(chunk - LAG)
                            gate_back(chunk - LAG, pA, pB, pig)

                    for cc in sorted(pend):
                        pA, pB, pig = pend[cc]
                        gate_back(cc, pA, pB, pig)
                    pend.clear()

                    replica(hnew, nc.sync)
                    if t + 1 < TT:
                        build_hB(hnew, hB[t % 2])
                    replica(Ft[t], nc.gpsimd)
                    if DBG:
                        nc.scalar.dma_start(d_dbg[layer][:][t, :, :],
                                            Ft[t][64:128, 0:FRAME])

            # trailing conv3d frame (T-1)
            for chunk in range(NCHUNK):
                conv3d_frame_chunk(TT - 1, chunk)

            # ================= final: bd matmul + sigmoid =================
            for chunk in range(NCHUNK):
                pY = pc3.tile([TT, CHUNK], F32, tag="pP", name="pY")
                nc.tensor.matmul(
                    pY[:], bdt[:],
                    ring[:, chunk * CHUNK:(chunk + 1) * CHUNK],
                    start=True, stop=True)
                ystg = gates.tile([TT, CHUNK], F32, tag="ystg", name="ystg",
                                  bufs=1)
                nc.scalar.activation(ystg[:], pY[:], AF.Sigmoid, bias=b3t[:])
                nc.sync.dma_start(
                    d_y[:][:, chunk * CHUNK:(chunk + 1) * CHUNK], ystg[:])

    nc.compile()
    return nc


def prep_inputs(x, k0, rk0, b0, g0, be0, k1, rk1, b1, g1, be1,
                k2, rk2, b2, g2, be2, w3, b3, TT=T):
    x = np.asarray(x, np.float32)
    wti, wtr = pack_weights(
        [np.asarray(k0, np.float32), np.asarray(k1, np.float32),
         np.asarray(k2, np.float32)],
        [np.asarray(rk0, np.float32), np.asarray(rk1, np.float32),
         np.asarray(rk2, np.float32)])
    w3t = pack_w3(np.asarray(w3, np.float32))
    b_all = np.stack([np.asarray(b0, np.float32),
                      np.asarray(b1, np.float32),
                      np.asarray(b2, np.float32)])
    scale = np.float32(1.0 / np.sqrt(1.0 + 1e-3))
    gb_all = np.stack([np.asarray(g0, np.float32) * scale,
                       np.asarray(g1, np.float32) * scale,
                       np.asarray(g2, np.float32) * scale])
    be_all = np.stack([np.asarray(be0, np.float32),
                       np.asarray(be1, np.float32),
                       np.asarray(be2, np.float32)])
    bd = np.zeros((3 * TT, TT), np.float32)
    for m in range(3):
        for t in range(TT):
            bd[m * TT + t, t] = 1.0
    b3b = np.full((TT, 1), np.asarray(b3, np.float32).ravel()[0], np.float32)

    shared = dict(wti=wti, wtr=wtr, w3t=w3t, bd=bd.astype(BF16),
                  b_all=b_all, gb_all=gb_all, be_all=be_all, b3b=b3b)
    in_maps = []
    for bb in range(B):
        xi = np.zeros((1, XLEN), BF16)
        fr = np.zeros((TT, PW, PW), np.float32)
        fr[:, 1:H + 1, 1:W + 1] = x[bb, :TT, :, :, 0]
        xi[0, XLEAD:XLEAD + TT * FRAME] = fr.reshape(-1).astype(BF16)
        m = dict(shared)
        m["x_im"] = xi
        in_maps.append(m)
    return in_maps


_CACHED = {}


def kernel(**inputs):
    from concourse.bass_utils import run_bass_kernel_spmd
    if 'nc' not in _CACHED:
        _CACHED['nc'] = build_nc(T)
    nc = _CACHED['nc']
    in_maps = prep_inputs(**inputs)
    res = run_bass_kernel_spmd(nc, in_maps, core_ids=list(range(B)),
                               trace=bool(os.environ.get('KTRACE')))
    _CACHED['last_res'] = res
    y = np.stack([r["y"].reshape(T, H, W, 1) for r in res.results])
    return y
